# revision 35
# baseline (speedup 1.0000x reference)
"""Trainium2 Bass kernel for nn_Diagnet (S=1024, B=64, I=512, H=2048, O=512).

    u = einsum('sbi,hi->sbh', X, W_ih)
    h_t = |u_t + hh * h_{t-1}|   (scan over S, only final h needed)
    Y = h_final @ W_ho.T + b_ho

Strategy (8 NeuronCores, data-parallel over batch, BC=8 rows per core):

* H lanes are permuted so hh is sorted descending and split into 16
  chunks of 128.  A chunk whose largest decay a satisfies a^K < tol
  only needs the last K steps (exact to ~tol relative), so each chunk
  gets a window K_g (multiple of 64), and the GEMM + scan skip
  everything earlier.
* The recurrence is computed by a custom DVE instruction that folds
  the WHOLE window in one go: out[tau] = |out[tau-1] - u[tau]*scn[tau]|
  via scan(ABSOLUTE_DIFF, Src0*Src1).  The running state lives in the
  engine (no SBUF round-trip per step), so the serial chain that
  dominated the naive per-step implementation (~200ns x 1024 steps)
  collapses to one ~K-cycle streaming instruction per (chunk, batch).
  scn[tau] = -a_lane^(K-1-tau) folds the per-step decay multiply into
  a prescale (a>=0 lets a*|x| = |a x|), and the minus sign turns
  ABSOLUTE_DIFF into abs-add.  h_final = last scan element (scale 1).
* GEMM runs in bf16 (1 cycle/row on the PE vs 4 for fp32; X DMA
  halves).  X is host-tiled to [block, i-chunk, 128i, (b,tau)] and
  kept resident in SBUF; the GEMM iterates chunk-major (longest
  window first) so each chunk's scan overlaps later chunks' GEMMs,
  with i-chunk-outer PSUM accumulation runs to amortize LDWEIGHTS.
* The Activation engine drains PSUM -> SBUF with a pure layout copy
  (to b-major contiguous windows); GPSIMD extracts h_final columns
  (cast to bf16); the final projection is 16 accumulating bf16
  matmuls + bias add at the end.
"""

import math
import os

from contextlib import ExitStack

import numpy as np

S, B, I, H, O = 1024, 64, 512, 2048, 512
NCORES = 8
BC = B // NCORES  # 8 batch rows per core
TB = 64  # block granularity for truncation windows
NBLK = S // TB  # 16
NCH = H // 128  # 16 h-chunks
NI = I // 128  # 4 i-chunks
USMALL_W = 256  # max window (cols) for chunks g>=1; K_1 <= 256 needs LN <= ~16

_CACHE = {}


def _register_scan_ops():
    """Two fold ops: m[t] = |m[t-1] - in0[t]*in1[t]|, seeded with 0 or with a
    per-partition value (s0) for chaining segment scans."""
    import concourse.dve_ops as dve_ops
    from concourse.dve_spec import C0, Spec, Src0, Src1, Zero, scan, lower, AluOp
    from concourse.dve_uop import DveOpSpec

    have = {op.name: op for op in dve_ops.OPS}
    if "ABSDIFF_SCALE_SCAN_ANT" in have:
        return have["ABSDIFF_SCALE_SCAN_ANT"], have["ABSDIFF_SCALE_SCAN_SEED_ANT"]

    def _ref_factory(seeded):
        def _ref(in0, in1, s0, s1, imm2):
            x = in0.astype(np.float32) * in1.astype(np.float32)
            out = np.empty_like(x)
            m = (
                np.broadcast_to(np.asarray(s0, np.float32).reshape(-1), (x.shape[0],))
                if seeded
                else np.zeros(x.shape[0], np.float32)
            ).copy()
            for t in range(x.shape[1]):
                m = np.abs(m - x[:, t])
                out[:, t] = m
            return out

        return _ref

    ops = []
    for name, init, seeded in (
        ("ABSDIFF_SCALE_SCAN_ANT", Zero, False),
        ("ABSDIFF_SCALE_SCAN_SEED_ANT", C0, True),
    ):
        spec = Spec(
            body=scan(AluOp.ABSOLUTE_DIFF, Src0 * Src1, init=init),
            reference=_ref_factory(seeded),
        )
        row = max(dve_ops._SUB_OPCODE_FOR_NAME.values()) + 1
        assert row < 0x20
        shas = {}
        for ver in ("v3", "v4"):
            s = DveOpSpec(name=name, opcode=row, uops=lower(spec, ver=ver), rd1_en=True)
            shas[ver] = s.sha(ver)
        op = dve_ops.DveOp(name, spec, subdim=False, uops_sha=shas)
        dve_ops._SUB_OPCODE_FOR_NAME[name] = row
        dve_ops.OPS.append(op)
        dve_ops.CUSTOM_DVE_SPECS[name] = spec
        ops.append(op)
    return ops[0], ops[1]


def _make_plan(hh):
    ln = float(os.environ.get("DIAG_LN", "9.2"))  # a^K <= e^-ln truncation tol
    a = np.maximum(np.abs(hh.astype(np.float64)), 1e-30)
    perm = np.argsort(-a, kind="stable")
    ag = a[perm].reshape(NCH, 128)  # [chunk, lane], descending
    windows = []
    for g in range(NCH):
        amax = ag[g, 0]
        if S * math.log(amax) >= -ln:
            kg = S
        else:
            kg = int(math.ceil(ln / math.log(1.0 / amax)))
        kg = min(S, max(TB, ((kg + TB - 1) // TB) * TB))
        windows.append(kg)
    assert all(windows[g] >= windows[g + 1] for g in range(NCH - 1)), windows
    assert all(k <= USMALL_W for k in windows[1:]), (windows, "raise USMALL_W")
    # chunk 0: K cols.  chunks g>=1: K+2 cols, the extra two being the
    # batch-separator scales (-1, +1) for the fused multi-batch scan.
    widths = [windows[0]] + [k + 2 for k in windows[1:]]
    offs = np.concatenate([[0], np.cumsum(widths)]).astype(int)
    scn = np.zeros((128, offs[-1]), dtype=np.float64)
    for g in range(NCH):
        kg = windows[g]
        tau = np.arange(kg)
        scn[:, offs[g] : offs[g] + kg] = -(ag[g][:, None] ** (kg - 1 - tau)[None, :])
        if g >= 1:
            scn[:, offs[g] + kg] = -1.0
            scn[:, offs[g] + kg + 1] = 1.0
    return {
        "perm": perm,
        "windows": tuple(windows),
        "offs": offs,
        "SCN": scn,  # float64; cast at the call site
    }


def _build(windows, offs_total):
    import concourse.mybir as mybir
    import concourse.tile as tile
    from concourse import bacc
    from concourse.bass import ds

    SCAN_OP, SCAN_SEED_OP = _register_scan_ops()
    f32 = mybir.dt.float32
    bf16 = mybir.dt.bfloat16
    R = int(os.environ.get("DIAG_R", "6"))

    nc = bacc.Bacc("TRN2", target_bir_lowering=False, debug=False, num_devices=NCORES)
    # X block layout: partition p (= i within chunk), line [ic, b, tau] (4KB bf16)
    X = nc.dram_tensor("X", [NBLK, 128, NI * TB * BC], bf16, kind="ExternalInput").ap()
    # WIHT line: [g, ic, hsub] (per-chunk contiguous pieces); WHOT line: [g, o]
    WIHT = nc.dram_tensor("WIHT", [128, NCH * NI * 128], bf16, kind="ExternalInput").ap()
    WHOT = nc.dram_tensor("WHOT", [128, NCH * O], bf16, kind="ExternalInput").ap()
    SCN = nc.dram_tensor("SCN", [128, offs_total], bf16, kind="ExternalInput").ap()
    BIAS = nc.dram_tensor("BIAS", [BC, O], f32, kind="ExternalInput").ap()
    Y = nc.dram_tensor("Y", [BC, O], f32, kind="ExternalOutput").ap()

    widths = [windows[0]] + [k + 2 for k in windows[1:]]
    offs = np.concatenate([[0], np.cumsum(widths)]).astype(int)

    with tile.TileContext(nc) as tc:
        with ExitStack() as ctx:
            consts = ctx.enter_context(tc.tile_pool(name="consts", bufs=1))
            xpool = ctx.enter_context(tc.tile_pool(name="xt", bufs=1))
            ubig = ctx.enter_context(tc.tile_pool(name="ubig", bufs=1))
            usmall = ctx.enter_context(tc.tile_pool(name="usmall", bufs=6))
            ypool = ctx.enter_context(tc.tile_pool(name="yout", bufs=1))
            gpool = ctx.enter_context(tc.tile_pool(name="gpsum", bufs=7, space="PSUM"))
            fpool = ctx.enter_context(tc.tile_pool(name="fpsum", bufs=1, space="PSUM"))

            # --- inputs.  Consumption order: chunks 15..1 (need only the last
            # 1-3 X blocks + their WIHT pieces), then chunk 0 which scans
            # blocks 0..15 in ascending time order.  X arrival order matches:
            # 15,14,13 first, then 0,1,2,...,12, split across the two HWDGE
            # queues (sync + scalar). ---
            wiht_t = consts.tile([128, NCH * NI * 128], bf16, tag="wiht", name="wiht_t")
            scn_t = consts.tile([128, offs_total], bf16, tag="scn", name="scn_t")
            xt = [
                xpool.tile([128, NI * TB * BC], bf16, tag=f"x{kb}", name=f"x_{kb}")
                for kb in range(NBLK)
            ]
            # DMA split across both HWDGE queues (SP + ACT) so descriptor
            # generation runs in parallel; ACT's queue drains well before its
            # first COPY is ready to run.
            def wp(g0, ng):  # wiht piece slice
                return ds(g0 * NI * 128, ng * NI * 128)

            nc.sync.dma_start(wiht_t[:, wp(NCH - 1, 1)], WIHT[:, wp(NCH - 1, 1)])
            nc.sync.dma_start(xt[NBLK - 1][:], X[NBLK - 1])
            nc.scalar.dma_start(xt[NBLK - 2][:], X[NBLK - 2])
            nc.sync.dma_start(wiht_t[:, wp(NCH - 3, 2)], WIHT[:, wp(NCH - 3, 2)])
            nc.scalar.dma_start(wiht_t[:, wp(6, 7)], WIHT[:, wp(6, 7)])
            nc.sync.dma_start(xt[NBLK - 3][:], X[NBLK - 3])
            nc.sync.dma_start(wiht_t[:, wp(0, 6)], WIHT[:, wp(0, 6)])
            nc.scalar.dma_start(scn_t[:], SCN)
            # chunk-0 blocks in ascending (scan) order, alternating queues
            for kb in range(0, NBLK - 3):
                (nc.sync if kb % 2 == 0 else nc.scalar).dma_start(xt[kb][:], X[kb])
            bias_t = ypool.tile([BC, O], f32, tag="bias", name="bias_t")
            nc.sync.dma_start(bias_t[:], BIAS)
            whot_t = consts.tile([128, NCH * O], bf16, tag="whot", name="whot_t")
            nc.scalar.dma_start(whot_t[:], WHOT)

            h_all = consts.tile([128, NCH * BC], bf16, tag="hall", name="h_all")

            # PE warm-up: dependency-free matmuls at t=0 lift the HAM clock
            # gate to 8/8 before the first real matmul arrives (~3.4us window)
            warm = consts.tile([128, TB * BC], f32, tag="warm", name="warm")
            nc.gpsimd.memset(warm[:], 0.0)
            wps = gpool.tile([128, TB * BC], f32, tag="gp", name="warm_ps")
            NWARM = 10
            for i in range(NWARM):
                nc.tensor.matmul(
                    wps[:],
                    warm[:, ds(0, 128)],
                    warm[:],
                    start=(i == 0),
                    stop=(i == NWARM - 1),
                )
            nc.scalar.copy(warm[:], wps[:])  # consume so the tiles are live

            # --- chunk-major pipeline: GEMM (PE) -> copy (ACT) -> scan (DVE) ---
            chunk_order = list(range(NCH - 1, 0, -1)) + [0]
            for g in chunk_order:
                kg = windows[g]
                nbg = kg // TB
                fb = NBLK - nbg
                if g == 0:
                    u_t = ubig.tile([128, BC * kg], f32, tag="u0", name="u_g0")
                    u3 = u_t[:].rearrange("p (b t) -> p b t", b=BC)
                else:
                    # per-batch width kg+2: the last two columns are the BIG
                    # separator pair that resets the fused scan between rows
                    u_t = usmall.tile(
                        [128, BC * (USMALL_W + 2)], f32, tag="us", name=f"u_g{g}"
                    )
                    u3 = u_t[:, ds(0, BC * (kg + 2))].rearrange(
                        "p (b t) -> p b t", b=BC
                    )
                    nc.gpsimd.memset(u3[:, :, ds(kg, 2)], 1.0e30)
                # chunk 0 consumes blocks in ascending (scan) order so each
                # GEMM run's segment scan chains off the previous one; other
                # chunks take newest-first (their X arrives first).
                blocks = (
                    list(range(fb, NBLK))
                    if g == 0
                    else list(range(NBLK - 1, fb - 1, -1))
                )
                def emit_gemm_copy(run):
                    ps = {
                        kb: gpool.tile([128, TB * BC], f32, tag="gp", name=f"gp_{g}_{kb}")
                        for kb in run
                    }
                    for ic in range(NI):
                        for kb in run:
                            nc.tensor.matmul(
                                ps[kb][:],
                                wiht_t[:, ds(g * NI * 128 + ic * 128, 128)],
                                xt[kb][:, ds(ic * TB * BC, TB * BC)],
                                start=(ic == 0),
                                stop=(ic == NI - 1),
                            )
                    for kb in run:
                        j = kb - fb
                        dst = u3[:, :, ds(j * TB, TB)]
                        src = ps[kb][:].rearrange("p (b t) -> p b t", b=BC)
                        nc.scalar.copy(dst, src)

                def emit_seg_scans(first_blk, n_blk):
                    # segment scan seeded by the previous segment's last
                    # element per (lane, b); first segment seeds with zero
                    seg0 = (first_blk - fb) * TB
                    seg = n_blk * TB
                    scn_s = scn_t[:, ds(int(offs[g]) + seg0, seg)]
                    for b in range(BC):
                        ap = u_t[:, ds(b * kg + seg0, seg)]
                        if seg0 == 0:
                            nc.vector._custom_dve(SCAN_OP, out=ap, in0=ap, in1=scn_s)
                        else:
                            seed = u_t[:, ds(b * kg + seg0 - 1, 1)]
                            nc.vector._custom_dve(
                                SCAN_SEED_OP, out=ap, in0=ap, in1=scn_s, s0=seed
                            )

                if g == 0:
                    # Full runs of R while >4 blocks remain; the last 4 blocks
                    # [w, x, y, z] GEMM as [x, y, z] (X already resident) then
                    # [w] (the last DMA arrival), so the post-DMA tail is only
                    # G+C of w, scan of [w], scan of [x, y, z].
                    full, left = [], list(blocks)
                    while len(left) > 4:
                        full.append(left[:R])
                        left = left[R:]
                    for run in full:
                        emit_gemm_copy(run)
                        emit_seg_scans(run[0], len(run))
                    if len(left) > 1:
                        emit_gemm_copy(left[1:])
                        emit_gemm_copy(left[:1])
                        emit_seg_scans(left[0], 1)
                        emit_seg_scans(left[1], len(left) - 1)
                    else:
                        emit_gemm_copy(left)
                        emit_seg_scans(left[0], 1)
                else:
                    for rs in range(0, len(blocks), R):
                        emit_gemm_copy(blocks[rs : rs + R])
                if g != 0:
                    # one fused scan over all batch rows: the (-1, +1)-scaled
                    # BIG separator pair exactly zeroes the state between rows
                    scn_g = (
                        scn_t[:, ds(int(offs[g]), kg + 2)]
                        .rearrange("p (o t) -> p o t", o=1)
                        .broadcast_to([128, BC, kg + 2])
                    )
                    nc.vector._custom_dve(SCAN_OP, out=u3, in0=u3, in1=scn_g)
                # h_final = last scan element per (lane, b) -> bf16
                hsrc = u3[:, :, kg - 1]
                nc.vector.tensor_copy(h_all[:, ds(g * BC, BC)], hsrc)

            # --- final projection: Y = h^T @ WHOT + bias ---
            # (emitted after all main-GEMM matmuls so no PE-FIFO stall; chunk 0
            # last, so the tail after its scan is a single matmul)
            psy = fpool.tile([BC, O], f32, tag="fy", name="psy")
            for i, g in enumerate(chunk_order):
                nc.tensor.matmul(
                    psy[:],
                    h_all[:, ds(g * BC, BC)],
                    whot_t[:, ds(g * O, O)],
                    start=(i == 0),
                    stop=(i == NCH - 1),
                )
            y_t = ypool.tile([BC, O], f32, tag="y", name="y_t")
            nc.vector.tensor_tensor(y_t[:], psy[:], bias_t[:], mybir.AluOpType.add)
            nc.sync.dma_start(Y, y_t[:])
    nc.compile()
    return nc


def _get_program(windows, offs_total):
    key = (
        windows,
        os.environ.get("DIAG_R"),
        os.environ.get("DIAG_LN"),
    )
    if key not in _CACHE:
        _CACHE[key] = _build(windows, offs_total)
    return _CACHE[key]


def _ensure_ntff_hook():
    """Provide antenv.axon_hooks (absent in this image) so trace=True works."""
    import sys
    import types

    if "antenv.axon_hooks" in sys.modules:
        return True
    try:
        import antenv

        mod = types.ModuleType("antenv.axon_hooks")
        mod._hook = None

        def set_axon_ntff_profile_hook(h):
            mod._hook = h

        def get_axon_ntff_profile_hook():
            return mod._hook

        mod.set_axon_ntff_profile_hook = set_axon_ntff_profile_hook
        mod.get_axon_ntff_profile_hook = get_axon_ntff_profile_hook
        sys.modules["antenv.axon_hooks"] = mod
        antenv.axon_hooks = mod

        from trn_agent_boot.trn_boot import _ntff_profile_via_ctypes

        hook = _ntff_profile_via_ctypes("/opt/axon/libaxon_pjrt.so")
        mod.set_axon_ntff_profile_hook(hook)
        return hook is not None
    except Exception:
        return False


def kernel(X, W_ih, hh, W_ho, b_ho):
    import ml_dtypes

    from concourse import bass_utils

    X = np.asarray(X, dtype=np.float32)
    W_ih = np.asarray(W_ih, dtype=np.float32)
    hh = np.asarray(hh, dtype=np.float32)
    W_ho = np.asarray(W_ho, dtype=np.float32)
    b_ho = np.asarray(b_ho, dtype=np.float32)

    plan = _make_plan(hh)
    perm = plan["perm"]
    nc = _get_program(plan["windows"], int(plan["offs"][-1]))

    bf = ml_dtypes.bfloat16
    # WIHT [128, NCH*NI*128]: line p = [g, ic, hsub], W_ih[h=g*128+hsub, i=ic*128+p]
    wiht = np.ascontiguousarray(
        W_ih[perm].T.reshape(NI, 128, NCH, 128).transpose(1, 2, 0, 3).reshape(128, -1)
    ).astype(bf)
    # WHOT [128, NCH*O]: line p = [g, o] with value W_ho[o, h=g*128+p]
    whot = np.ascontiguousarray(
        W_ho[:, perm].T.reshape(NCH, 128, O).transpose(1, 0, 2).reshape(128, NCH * O)
    ).astype(bf)
    bias = np.tile(b_ho[None, :], (BC, 1)).astype(np.float32)

    common = {
        "WIHT": wiht,
        "WHOT": whot,
        "BIAS": bias,
        "SCN": plan["SCN"].astype(bf),
    }
    in_maps = []
    for m in range(NCORES):
        im = dict(common)
        xm = X[:, m * BC : (m + 1) * BC, :]  # [S, BC, I]
        # device layout [NBLK, 128(i-in-chunk), (ic, b, tau)]
        xt = xm.transpose(2, 1, 0).reshape(NI, 128, BC, NBLK, TB)
        xt = np.ascontiguousarray(xt.transpose(3, 1, 0, 2, 4)).reshape(
            NBLK, 128, NI * BC * TB
        )
        im["X"] = xt.astype(bf)
        in_maps.append(im)

    trace = bool(int(os.environ.get("DIAG_TRACE", "0")))
    if trace:
        trace = _ensure_ntff_hook()
    res = None
    for attempt in range(3):
        try:
            res = bass_utils.run_bass_kernel_spmd(
                nc,
                in_maps,
                core_ids=list(range(NCORES)),
                trace=trace,
                tmpdir=os.environ.get("DIAG_TRACE_DIR") or None,
            )
            break
        except Exception:
            if attempt == 2:
                raise
            trace = False  # retry without profiling
    if res.exec_time_ns is not None:
        kernel.last_exec_time_ns = res.exec_time_ns
        kernel.last_mean_exec_time_ns = res.mean_exec_time_ns
    Yfull = np.concatenate([r["Y"] for r in res.results], axis=0)
    return Yfull


kernel.last_exec_time_ns = None
kernel.last_mean_exec_time_ns = None


# revision 36
# speedup vs baseline: 1.1129x; 1.1129x over previous
"""Trainium2 Bass kernel for nn_Diagnet (S=1024, B=64, I=512, H=2048, O=512).

    u = einsum('sbi,hi->sbh', X, W_ih)
    h_t = |u_t + hh * h_{t-1}|   (scan over S, only final h needed)
    Y = h_final @ W_ho.T + b_ho

Strategy (8 NeuronCores, data-parallel over batch, BC=8 rows per core):

* H lanes are permuted so hh is sorted descending and split into 16
  chunks of 128.  A chunk whose largest decay a satisfies a^K < tol
  only needs the last K steps (exact to ~tol relative), so each chunk
  gets a window K_g (multiple of 64), and the GEMM + scan skip
  everything earlier.
* The recurrence is computed by a custom DVE instruction that folds
  the WHOLE window in one go: out[tau] = |out[tau-1] - u[tau]*scn[tau]|
  via scan(ABSOLUTE_DIFF, Src0*Src1).  The running state lives in the
  engine (no SBUF round-trip per step), so the serial chain that
  dominated the naive per-step implementation (~200ns x 1024 steps)
  collapses to one ~K-cycle streaming instruction per (chunk, batch).
  scn[tau] = -a_lane^(K-1-tau) folds the per-step decay multiply into
  a prescale (a>=0 lets a*|x| = |a x|), and the minus sign turns
  ABSOLUTE_DIFF into abs-add.  h_final = last scan element (scale 1).
* GEMM runs in bf16 (1 cycle/row on the PE vs 4 for fp32; X DMA
  halves).  X is host-tiled to [block, i-chunk, 128i, (b,tau)] and
  kept resident in SBUF; the GEMM iterates chunk-major (longest
  window first) so each chunk's scan overlaps later chunks' GEMMs,
  with i-chunk-outer PSUM accumulation runs to amortize LDWEIGHTS.
* The Activation engine drains PSUM -> SBUF with a pure layout copy
  (to b-major contiguous windows); GPSIMD extracts h_final columns
  (cast to bf16); the final projection is 16 accumulating bf16
  matmuls + bias add at the end.
"""

import math
import os

from contextlib import ExitStack

import numpy as np

S, B, I, H, O = 1024, 64, 512, 2048, 512
NCORES = 8
BC = B // NCORES  # 8 batch rows per core
TB = 64  # block granularity for truncation windows
NBLK = S // TB  # 16
NCH = H // 128  # 16 h-chunks
NI = I // 128  # 4 i-chunks
USMALL_W = 256  # max window (cols) for chunks g>=1; K_1 <= 256 needs LN <= ~16

_CACHE = {}


def _register_scan_ops():
    """Two fold ops: m[t] = |m[t-1] - in0[t]*in1[t]|, seeded with 0 or with a
    per-partition value (s0) for chaining segment scans."""
    import concourse.dve_ops as dve_ops
    from concourse.dve_spec import C0, Spec, Src0, Src1, Zero, scan, lower, AluOp
    from concourse.dve_uop import DveOpSpec

    have = {op.name: op for op in dve_ops.OPS}
    if "ABSDIFF_SCALE_SCAN_ANT" in have:
        return have["ABSDIFF_SCALE_SCAN_ANT"], have["ABSDIFF_SCALE_SCAN_SEED_ANT"]

    def _ref_factory(seeded):
        def _ref(in0, in1, s0, s1, imm2):
            x = in0.astype(np.float32) * in1.astype(np.float32)
            out = np.empty_like(x)
            m = (
                np.broadcast_to(np.asarray(s0, np.float32).reshape(-1), (x.shape[0],))
                if seeded
                else np.zeros(x.shape[0], np.float32)
            ).copy()
            for t in range(x.shape[1]):
                m = np.abs(m - x[:, t])
                out[:, t] = m
            return out

        return _ref

    ops = []
    for name, init, seeded in (
        ("ABSDIFF_SCALE_SCAN_ANT", Zero, False),
        ("ABSDIFF_SCALE_SCAN_SEED_ANT", C0, True),
    ):
        spec = Spec(
            body=scan(AluOp.ABSOLUTE_DIFF, Src0 * Src1, init=init),
            reference=_ref_factory(seeded),
        )
        row = max(dve_ops._SUB_OPCODE_FOR_NAME.values()) + 1
        assert row < 0x20
        shas = {}
        for ver in ("v3", "v4"):
            s = DveOpSpec(name=name, opcode=row, uops=lower(spec, ver=ver), rd1_en=True)
            shas[ver] = s.sha(ver)
        op = dve_ops.DveOp(name, spec, subdim=False, uops_sha=shas)
        dve_ops._SUB_OPCODE_FOR_NAME[name] = row
        dve_ops.OPS.append(op)
        dve_ops.CUSTOM_DVE_SPECS[name] = spec
        ops.append(op)
    return ops[0], ops[1]


def _make_plan(hh):
    ln = float(os.environ.get("DIAG_LN", "9.2"))  # a^K <= e^-ln truncation tol
    a = np.maximum(np.abs(hh.astype(np.float64)), 1e-30)
    perm = np.argsort(-a, kind="stable")
    ag = a[perm].reshape(NCH, 128)  # [chunk, lane], descending
    windows = []
    for g in range(NCH):
        amax = ag[g, 0]
        if S * math.log(amax) >= -ln:
            kg = S
        else:
            kg = int(math.ceil(ln / math.log(1.0 / amax)))
        kg = min(S, max(TB, ((kg + TB - 1) // TB) * TB))
        windows.append(kg)
    assert all(windows[g] >= windows[g + 1] for g in range(NCH - 1)), windows
    assert all(k <= USMALL_W for k in windows[1:]), (windows, "raise USMALL_W")
    # chunk 0: K cols.  chunks g>=1: K+2 cols, the extra two being the
    # batch-separator scales (-1, +1) for the fused multi-batch scan.
    widths = [windows[0]] + [k + 2 for k in windows[1:]]
    offs = np.concatenate([[0], np.cumsum(widths)]).astype(int)
    scn = np.zeros((128, offs[-1]), dtype=np.float64)
    for g in range(NCH):
        kg = windows[g]
        tau = np.arange(kg)
        scn[:, offs[g] : offs[g] + kg] = -(ag[g][:, None] ** (kg - 1 - tau)[None, :])
        if g >= 1:
            scn[:, offs[g] + kg] = -1.0
            scn[:, offs[g] + kg + 1] = 1.0
    return {
        "perm": perm,
        "windows": tuple(windows),
        "offs": offs,
        "SCN": scn,  # float64; cast at the call site
    }


def _build(windows, offs_total):
    import concourse.mybir as mybir
    import concourse.tile as tile
    from concourse import bacc
    from concourse.bass import ds

    SCAN_OP, SCAN_SEED_OP = _register_scan_ops()
    f32 = mybir.dt.float32
    bf16 = mybir.dt.bfloat16
    R = int(os.environ.get("DIAG_R", "6"))

    nc = bacc.Bacc("TRN2", target_bir_lowering=False, debug=False, num_devices=NCORES)
    # X block layout: partition p (= i within chunk), line [ic, b, tau] (4KB bf16)
    X = nc.dram_tensor("X", [NBLK, 128, NI * TB * BC], bf16, kind="ExternalInput").ap()
    # WIHT line: [g, ic, hsub] (per-chunk contiguous pieces); WHOT line: [g, o]
    WIHT = nc.dram_tensor("WIHT", [128, NCH * NI * 128], bf16, kind="ExternalInput").ap()
    WHOT = nc.dram_tensor("WHOT", [128, NCH * O], bf16, kind="ExternalInput").ap()
    SCN = nc.dram_tensor("SCN", [128, offs_total], bf16, kind="ExternalInput").ap()
    BIAS = nc.dram_tensor("BIAS", [BC, O], f32, kind="ExternalInput").ap()
    Y = nc.dram_tensor("Y", [BC, O], f32, kind="ExternalOutput").ap()

    widths = [windows[0]] + [k + 2 for k in windows[1:]]
    offs = np.concatenate([[0], np.cumsum(widths)]).astype(int)

    with tile.TileContext(nc) as tc:
        with ExitStack() as ctx:
            consts = ctx.enter_context(tc.tile_pool(name="consts", bufs=1))
            xpool = ctx.enter_context(tc.tile_pool(name="xt", bufs=1))
            ubig = ctx.enter_context(tc.tile_pool(name="ubig", bufs=1))
            usmall = ctx.enter_context(tc.tile_pool(name="usmall", bufs=6))
            ypool = ctx.enter_context(tc.tile_pool(name="yout", bufs=1))
            gpool = ctx.enter_context(tc.tile_pool(name="gpsum", bufs=7, space="PSUM"))
            fpool = ctx.enter_context(tc.tile_pool(name="fpsum", bufs=1, space="PSUM"))

            # --- inputs.  Consumption order: chunks 15..1 (need only the last
            # 1-3 X blocks + their WIHT pieces), then chunk 0 which scans
            # blocks 0..15 in ascending time order.  X arrival order matches:
            # 15,14,13 first, then 0,1,2,...,12, split across the two HWDGE
            # queues (sync + scalar). ---
            wiht_t = consts.tile([128, NCH * NI * 128], bf16, tag="wiht", name="wiht_t")
            scn_t = consts.tile([128, offs_total], bf16, tag="scn", name="scn_t")
            xt = [
                xpool.tile([128, NI * TB * BC], bf16, tag=f"x{kb}", name=f"x_{kb}")
                for kb in range(NBLK)
            ]
            # DMA split across both HWDGE queues (SP + ACT) so descriptor
            # generation runs in parallel; ACT's queue drains well before its
            # first COPY is ready to run.
            def wp(g0, ng):  # wiht piece slice
                return ds(g0 * NI * 128, ng * NI * 128)

            nc.sync.dma_start(wiht_t[:, wp(NCH - 1, 1)], WIHT[:, wp(NCH - 1, 1)])
            nc.sync.dma_start(xt[NBLK - 1][:], X[NBLK - 1])
            nc.sync.dma_start(wiht_t[:, wp(NCH - 3, 2)], WIHT[:, wp(NCH - 3, 2)])
            nc.sync.dma_start(xt[NBLK - 2][:], X[NBLK - 2])
            nc.sync.dma_start(xt[NBLK - 3][:], X[NBLK - 3])
            nc.sync.dma_start(wiht_t[:, wp(0, NCH - 3)], WIHT[:, wp(0, NCH - 3)])
            nc.sync.dma_start(scn_t[:], SCN)
            # chunk-0 blocks in ascending (scan) order
            for kb in range(0, NBLK - 3):
                nc.sync.dma_start(xt[kb][:], X[kb])
            bias_t = ypool.tile([BC, O], f32, tag="bias", name="bias_t")
            nc.sync.dma_start(bias_t[:], BIAS)
            whot_t = consts.tile([128, NCH * O], bf16, tag="whot", name="whot_t")
            nc.sync.dma_start(whot_t[:], WHOT)

            h_all = consts.tile([128, NCH * BC], bf16, tag="hall", name="h_all")

            # PE warm-up: dependency-free matmuls at t=0 lift the HAM clock
            # gate to 8/8 before the first real matmul arrives (~3.4us window)
            warm = consts.tile([128, TB * BC], f32, tag="warm", name="warm")
            nc.gpsimd.memset(warm[:], 0.0)
            wps = gpool.tile([128, TB * BC], f32, tag="gp", name="warm_ps")
            NWARM = 10
            for i in range(NWARM):
                nc.tensor.matmul(
                    wps[:],
                    warm[:, ds(0, 128)],
                    warm[:],
                    start=(i == 0),
                    stop=(i == NWARM - 1),
                )
            nc.scalar.copy(warm[:], wps[:])  # consume so the tiles are live

            # --- chunk-major pipeline: GEMM (PE) -> copy (ACT) -> scan (DVE) ---
            chunk_order = list(range(NCH - 1, 0, -1)) + [0]
            for g in chunk_order:
                kg = windows[g]
                nbg = kg // TB
                fb = NBLK - nbg
                if g == 0:
                    u_t = ubig.tile([128, BC * kg], f32, tag="u0", name="u_g0")
                    u3 = u_t[:].rearrange("p (b t) -> p b t", b=BC)
                else:
                    # per-batch width kg+2: the last two columns are the BIG
                    # separator pair that resets the fused scan between rows
                    u_t = usmall.tile(
                        [128, BC * (USMALL_W + 2)], f32, tag="us", name=f"u_g{g}"
                    )
                    u3 = u_t[:, ds(0, BC * (kg + 2))].rearrange(
                        "p (b t) -> p b t", b=BC
                    )
                    nc.gpsimd.memset(u3[:, :, ds(kg, 2)], 1.0e30)
                # chunk 0 consumes blocks in ascending (scan) order so each
                # GEMM run's segment scan chains off the previous one; other
                # chunks take newest-first (their X arrives first).
                blocks = (
                    list(range(fb, NBLK))
                    if g == 0
                    else list(range(NBLK - 1, fb - 1, -1))
                )
                def emit_gemm_copy(run):
                    ps = {
                        kb: gpool.tile([128, TB * BC], f32, tag="gp", name=f"gp_{g}_{kb}")
                        for kb in run
                    }
                    for ic in range(NI):
                        for kb in run:
                            nc.tensor.matmul(
                                ps[kb][:],
                                wiht_t[:, ds(g * NI * 128 + ic * 128, 128)],
                                xt[kb][:, ds(ic * TB * BC, TB * BC)],
                                start=(ic == 0),
                                stop=(ic == NI - 1),
                            )
                    for kb in run:
                        j = kb - fb
                        dst = u3[:, :, ds(j * TB, TB)]
                        src = ps[kb][:].rearrange("p (b t) -> p b t", b=BC)
                        nc.scalar.copy(dst, src)

                def emit_seg_scans(first_blk, n_blk):
                    # segment scan seeded by the previous segment's last
                    # element per (lane, b); first segment seeds with zero
                    seg0 = (first_blk - fb) * TB
                    seg = n_blk * TB
                    scn_s = scn_t[:, ds(int(offs[g]) + seg0, seg)]
                    for b in range(BC):
                        ap = u_t[:, ds(b * kg + seg0, seg)]
                        if seg0 == 0:
                            nc.vector._custom_dve(SCAN_OP, out=ap, in0=ap, in1=scn_s)
                        else:
                            seed = u_t[:, ds(b * kg + seg0 - 1, 1)]
                            nc.vector._custom_dve(
                                SCAN_SEED_OP, out=ap, in0=ap, in1=scn_s, s0=seed
                            )

                if g == 0:
                    # Full runs of R while >4 blocks remain; the last 4 blocks
                    # [w, x, y, z] GEMM as [x, y, z] (X already resident) then
                    # [w] (the last DMA arrival), so the post-DMA tail is only
                    # G+C of w, scan of [w], scan of [x, y, z].
                    full, left = [], list(blocks)
                    while len(left) > 4:
                        full.append(left[:R])
                        left = left[R:]
                    for run in full:
                        emit_gemm_copy(run)
                        emit_seg_scans(run[0], len(run))
                    if len(left) > 1:
                        emit_gemm_copy(left[1:])
                        emit_gemm_copy(left[:1])
                        emit_seg_scans(left[0], 1)
                        emit_seg_scans(left[1], len(left) - 1)
                    else:
                        emit_gemm_copy(left)
                        emit_seg_scans(left[0], 1)
                else:
                    for rs in range(0, len(blocks), R):
                        emit_gemm_copy(blocks[rs : rs + R])
                if g != 0:
                    # one fused scan over all batch rows: the (-1, +1)-scaled
                    # BIG separator pair exactly zeroes the state between rows
                    scn_g = (
                        scn_t[:, ds(int(offs[g]), kg + 2)]
                        .rearrange("p (o t) -> p o t", o=1)
                        .broadcast_to([128, BC, kg + 2])
                    )
                    nc.vector._custom_dve(SCAN_OP, out=u3, in0=u3, in1=scn_g)
                # h_final = last scan element per (lane, b) -> bf16
                hsrc = u3[:, :, kg - 1]
                nc.vector.tensor_copy(h_all[:, ds(g * BC, BC)], hsrc)

            # --- final projection: Y = h^T @ WHOT + bias ---
            # (emitted after all main-GEMM matmuls so no PE-FIFO stall; chunk 0
            # last, so the tail after its scan is a single matmul)
            psy = fpool.tile([BC, O], f32, tag="fy", name="psy")
            for i, g in enumerate(chunk_order):
                nc.tensor.matmul(
                    psy[:],
                    h_all[:, ds(g * BC, BC)],
                    whot_t[:, ds(g * O, O)],
                    start=(i == 0),
                    stop=(i == NCH - 1),
                )
            y_t = ypool.tile([BC, O], f32, tag="y", name="y_t")
            nc.vector.tensor_tensor(y_t[:], psy[:], bias_t[:], mybir.AluOpType.add)
            nc.sync.dma_start(Y, y_t[:])
    nc.compile()
    return nc


def _get_program(windows, offs_total):
    key = (
        windows,
        os.environ.get("DIAG_R"),
        os.environ.get("DIAG_LN"),
    )
    if key not in _CACHE:
        _CACHE[key] = _build(windows, offs_total)
    return _CACHE[key]


def _ensure_ntff_hook():
    """Provide antenv.axon_hooks (absent in this image) so trace=True works."""
    import sys
    import types

    if "antenv.axon_hooks" in sys.modules:
        return True
    try:
        import antenv

        mod = types.ModuleType("antenv.axon_hooks")
        mod._hook = None

        def set_axon_ntff_profile_hook(h):
            mod._hook = h

        def get_axon_ntff_profile_hook():
            return mod._hook

        mod.set_axon_ntff_profile_hook = set_axon_ntff_profile_hook
        mod.get_axon_ntff_profile_hook = get_axon_ntff_profile_hook
        sys.modules["antenv.axon_hooks"] = mod
        antenv.axon_hooks = mod

        from trn_agent_boot.trn_boot import _ntff_profile_via_ctypes

        hook = _ntff_profile_via_ctypes("/opt/axon/libaxon_pjrt.so")
        mod.set_axon_ntff_profile_hook(hook)
        return hook is not None
    except Exception:
        return False


def kernel(X, W_ih, hh, W_ho, b_ho):
    import ml_dtypes

    from concourse import bass_utils

    X = np.asarray(X, dtype=np.float32)
    W_ih = np.asarray(W_ih, dtype=np.float32)
    hh = np.asarray(hh, dtype=np.float32)
    W_ho = np.asarray(W_ho, dtype=np.float32)
    b_ho = np.asarray(b_ho, dtype=np.float32)

    plan = _make_plan(hh)
    perm = plan["perm"]
    nc = _get_program(plan["windows"], int(plan["offs"][-1]))

    bf = ml_dtypes.bfloat16
    # WIHT [128, NCH*NI*128]: line p = [g, ic, hsub], W_ih[h=g*128+hsub, i=ic*128+p]
    wiht = np.ascontiguousarray(
        W_ih[perm].T.reshape(NI, 128, NCH, 128).transpose(1, 2, 0, 3).reshape(128, -1)
    ).astype(bf)
    # WHOT [128, NCH*O]: line p = [g, o] with value W_ho[o, h=g*128+p]
    whot = np.ascontiguousarray(
        W_ho[:, perm].T.reshape(NCH, 128, O).transpose(1, 0, 2).reshape(128, NCH * O)
    ).astype(bf)
    bias = np.tile(b_ho[None, :], (BC, 1)).astype(np.float32)

    common = {
        "WIHT": wiht,
        "WHOT": whot,
        "BIAS": bias,
        "SCN": plan["SCN"].astype(bf),
    }
    in_maps = []
    for m in range(NCORES):
        im = dict(common)
        xm = X[:, m * BC : (m + 1) * BC, :]  # [S, BC, I]
        # device layout [NBLK, 128(i-in-chunk), (ic, b, tau)]
        xt = xm.transpose(2, 1, 0).reshape(NI, 128, BC, NBLK, TB)
        xt = np.ascontiguousarray(xt.transpose(3, 1, 0, 2, 4)).reshape(
            NBLK, 128, NI * BC * TB
        )
        im["X"] = xt.astype(bf)
        in_maps.append(im)

    trace = bool(int(os.environ.get("DIAG_TRACE", "0")))
    if trace:
        trace = _ensure_ntff_hook()
    res = None
    for attempt in range(3):
        try:
            res = bass_utils.run_bass_kernel_spmd(
                nc,
                in_maps,
                core_ids=list(range(NCORES)),
                trace=trace,
                tmpdir=os.environ.get("DIAG_TRACE_DIR") or None,
            )
            break
        except Exception:
            if attempt == 2:
                raise
            trace = False  # retry without profiling
    if res.exec_time_ns is not None:
        kernel.last_exec_time_ns = res.exec_time_ns
        kernel.last_mean_exec_time_ns = res.mean_exec_time_ns
    Yfull = np.concatenate([r["Y"] for r in res.results], axis=0)
    return Yfull


kernel.last_exec_time_ns = None
kernel.last_mean_exec_time_ns = None


# revision 37
# speedup vs baseline: 1.1561x; 1.0388x over previous
"""Trainium2 Bass kernel for nn_Diagnet (S=1024, B=64, I=512, H=2048, O=512).

    u = einsum('sbi,hi->sbh', X, W_ih)
    h_t = |u_t + hh * h_{t-1}|   (scan over S, only final h needed)
    Y = h_final @ W_ho.T + b_ho

Strategy (8 NeuronCores, data-parallel over batch, BC=8 rows per core):

* H lanes are permuted so hh is sorted descending and split into 16
  chunks of 128.  A chunk whose largest decay a satisfies a^K < tol
  only needs the last K steps (exact to ~tol relative), so each chunk
  gets a window K_g (multiple of 64), and the GEMM + scan skip
  everything earlier.
* The recurrence is computed by a custom DVE instruction that folds
  the WHOLE window in one go: out[tau] = |out[tau-1] - u[tau]*scn[tau]|
  via scan(ABSOLUTE_DIFF, Src0*Src1).  The running state lives in the
  engine (no SBUF round-trip per step), so the serial chain that
  dominated the naive per-step implementation (~200ns x 1024 steps)
  collapses to one ~K-cycle streaming instruction per (chunk, batch).
  scn[tau] = -a_lane^(K-1-tau) folds the per-step decay multiply into
  a prescale (a>=0 lets a*|x| = |a x|), and the minus sign turns
  ABSOLUTE_DIFF into abs-add.  h_final = last scan element (scale 1).
* GEMM runs in bf16 (1 cycle/row on the PE vs 4 for fp32; X DMA
  halves).  X is host-tiled to [block, i-chunk, 128i, (b,tau)] and
  kept resident in SBUF; the GEMM iterates chunk-major (longest
  window first) so each chunk's scan overlaps later chunks' GEMMs,
  with i-chunk-outer PSUM accumulation runs to amortize LDWEIGHTS.
* The Activation engine drains PSUM -> SBUF with a pure layout copy
  (to b-major contiguous windows); GPSIMD extracts h_final columns
  (cast to bf16); the final projection is 16 accumulating bf16
  matmuls + bias add at the end.
"""

import math
import os

from contextlib import ExitStack

import numpy as np

S, B, I, H, O = 1024, 64, 512, 2048, 512
NCORES = 8
BC = B // NCORES  # 8 batch rows per core
TB = 64  # block granularity for truncation windows
NBLK = S // TB  # 16
NCH = H // 128  # 16 h-chunks
NI = I // 128  # 4 i-chunks
USMALL_W = 256  # max window (cols) for chunks g>=1; K_1 <= 256 needs LN <= ~16

_CACHE = {}


def _register_scan_ops():
    """Two fold ops: m[t] = |m[t-1] - in0[t]*in1[t]|, seeded with 0 or with a
    per-partition value (s0) for chaining segment scans."""
    import concourse.dve_ops as dve_ops
    from concourse.dve_spec import C0, Spec, Src0, Src1, Zero, scan, lower, AluOp
    from concourse.dve_uop import DveOpSpec

    have = {op.name: op for op in dve_ops.OPS}
    if "ABSDIFF_SCALE_SCAN_ANT" in have:
        return have["ABSDIFF_SCALE_SCAN_ANT"], have["ABSDIFF_SCALE_SCAN_SEED_ANT"]

    def _ref_factory(seeded):
        def _ref(in0, in1, s0, s1, imm2):
            x = in0.astype(np.float32) * in1.astype(np.float32)
            out = np.empty_like(x)
            m = (
                np.broadcast_to(np.asarray(s0, np.float32).reshape(-1), (x.shape[0],))
                if seeded
                else np.zeros(x.shape[0], np.float32)
            ).copy()
            for t in range(x.shape[1]):
                m = np.abs(m - x[:, t])
                out[:, t] = m
            return out

        return _ref

    ops = []
    for name, init, seeded in (
        ("ABSDIFF_SCALE_SCAN_ANT", Zero, False),
        ("ABSDIFF_SCALE_SCAN_SEED_ANT", C0, True),
    ):
        spec = Spec(
            body=scan(AluOp.ABSOLUTE_DIFF, Src0 * Src1, init=init),
            reference=_ref_factory(seeded),
        )
        row = max(dve_ops._SUB_OPCODE_FOR_NAME.values()) + 1
        assert row < 0x20
        shas = {}
        for ver in ("v3", "v4"):
            s = DveOpSpec(name=name, opcode=row, uops=lower(spec, ver=ver), rd1_en=True)
            shas[ver] = s.sha(ver)
        op = dve_ops.DveOp(name, spec, subdim=False, uops_sha=shas)
        dve_ops._SUB_OPCODE_FOR_NAME[name] = row
        dve_ops.OPS.append(op)
        dve_ops.CUSTOM_DVE_SPECS[name] = spec
        ops.append(op)
    return ops[0], ops[1]


def _make_plan(hh):
    ln = float(os.environ.get("DIAG_LN", "9.2"))  # a^K <= e^-ln truncation tol
    a = np.maximum(np.abs(hh.astype(np.float64)), 1e-30)
    perm = np.argsort(-a, kind="stable")
    ag = a[perm].reshape(NCH, 128)  # [chunk, lane], descending
    windows = []
    for g in range(NCH):
        amax = ag[g, 0]
        if S * math.log(amax) >= -ln:
            kg = S
        else:
            kg = int(math.ceil(ln / math.log(1.0 / amax)))
        kg = min(S, max(TB, ((kg + TB - 1) // TB) * TB))
        windows.append(kg)
    assert all(windows[g] >= windows[g + 1] for g in range(NCH - 1)), windows
    assert all(k <= USMALL_W for k in windows[1:]), (windows, "raise USMALL_W")
    # chunk 0: K cols.  chunks g>=1: K+2 cols, the extra two being the
    # batch-separator scales (-1, +1) for the fused multi-batch scan.
    widths = [windows[0]] + [k + 2 for k in windows[1:]]
    offs = np.concatenate([[0], np.cumsum(widths)]).astype(int)
    scn = np.zeros((128, offs[-1]), dtype=np.float64)
    for g in range(NCH):
        kg = windows[g]
        tau = np.arange(kg)
        scn[:, offs[g] : offs[g] + kg] = -(ag[g][:, None] ** (kg - 1 - tau)[None, :])
        if g >= 1:
            scn[:, offs[g] + kg] = -1.0
            scn[:, offs[g] + kg + 1] = 1.0
    return {
        "perm": perm,
        "windows": tuple(windows),
        "offs": offs,
        "SCN": scn,  # float64; cast at the call site
    }


def _build(windows, offs_total):
    import concourse.mybir as mybir
    import concourse.tile as tile
    from concourse import bacc
    from concourse.bass import ds

    SCAN_OP, SCAN_SEED_OP = _register_scan_ops()
    f32 = mybir.dt.float32
    bf16 = mybir.dt.bfloat16
    R = int(os.environ.get("DIAG_R", "6"))

    nc = bacc.Bacc("TRN2", target_bir_lowering=False, debug=False, num_devices=NCORES)
    # X block layout: partition p (= i within chunk), line [ic, b, tau] (4KB bf16)
    X = nc.dram_tensor("X", [NBLK, 128, NI * TB * BC], bf16, kind="ExternalInput").ap()
    # WIHT line: [g, ic, hsub] (per-chunk contiguous pieces); WHOT line: [g, o]
    WIHT = nc.dram_tensor("WIHT", [128, NCH * NI * 128], bf16, kind="ExternalInput").ap()
    WHOT = nc.dram_tensor("WHOT", [128, NCH * O], bf16, kind="ExternalInput").ap()
    SCN = nc.dram_tensor("SCN", [128, offs_total], bf16, kind="ExternalInput").ap()
    BIAS = nc.dram_tensor("BIAS", [BC, O], f32, kind="ExternalInput").ap()
    Y = nc.dram_tensor("Y", [BC, O], f32, kind="ExternalOutput").ap()

    widths = [windows[0]] + [k + 2 for k in windows[1:]]
    offs = np.concatenate([[0], np.cumsum(widths)]).astype(int)

    with tile.TileContext(nc) as tc:
        with ExitStack() as ctx:
            consts = ctx.enter_context(tc.tile_pool(name="consts", bufs=1))
            xpool = ctx.enter_context(tc.tile_pool(name="xt", bufs=1))
            ubig = ctx.enter_context(tc.tile_pool(name="ubig", bufs=1))
            usmall = ctx.enter_context(tc.tile_pool(name="usmall", bufs=6))
            ypool = ctx.enter_context(tc.tile_pool(name="yout", bufs=1))
            gpool = ctx.enter_context(tc.tile_pool(name="gpsum", bufs=7, space="PSUM"))
            fpool = ctx.enter_context(tc.tile_pool(name="fpsum", bufs=1, space="PSUM"))

            # --- inputs.  Consumption order: chunks 15..1 (need only the last
            # 1-3 X blocks + their WIHT pieces), then chunk 0 which scans
            # blocks 0..15 in ascending time order.  X arrival order matches:
            # 15,14,13 first, then 0,1,2,...,12, split across the two HWDGE
            # queues (sync + scalar). ---
            wiht_t = consts.tile([128, NCH * NI * 128], bf16, tag="wiht", name="wiht_t")
            scn_t = consts.tile([128, offs_total], bf16, tag="scn", name="scn_t")
            xt = [
                xpool.tile([128, NI * TB * BC], bf16, tag=f"x{kb}", name=f"x_{kb}")
                for kb in range(NBLK)
            ]
            # DMA split across both HWDGE queues (SP + ACT) so descriptor
            # generation runs in parallel; ACT's queue drains well before its
            # first COPY is ready to run.
            def wp(g0, ng):  # wiht piece slice
                return ds(g0 * NI * 128, ng * NI * 128)

            nc.sync.dma_start(wiht_t[:, wp(NCH - 1, 1)], WIHT[:, wp(NCH - 1, 1)])
            nc.sync.dma_start(xt[NBLK - 1][:], X[NBLK - 1])
            nc.sync.dma_start(wiht_t[:, wp(NCH - 3, 2)], WIHT[:, wp(NCH - 3, 2)])
            nc.sync.dma_start(xt[NBLK - 2][:], X[NBLK - 2])
            nc.sync.dma_start(xt[NBLK - 3][:], X[NBLK - 3])
            nc.sync.dma_start(wiht_t[:, wp(0, NCH - 3)], WIHT[:, wp(0, NCH - 3)])
            nc.sync.dma_start(scn_t[:], SCN)
            # chunk-0 blocks in ascending (scan) order
            for kb in range(0, NBLK - 3):
                nc.sync.dma_start(xt[kb][:], X[kb])
            bias_t = ypool.tile([BC, O], f32, tag="bias", name="bias_t")
            nc.sync.dma_start(bias_t[:], BIAS)
            whot_t = consts.tile([128, NCH * O], bf16, tag="whot", name="whot_t")
            nc.sync.dma_start(whot_t[:], WHOT)

            h_all = consts.tile([128, NCH * BC], bf16, tag="hall", name="h_all")

            # PE warm-up: dependency-free matmuls at t=0 lift the HAM clock
            # gate to 8/8 before the first real matmul arrives (~3.4us window)
            warm = consts.tile([128, TB * BC], f32, tag="warm", name="warm")
            nc.gpsimd.memset(warm[:], 0.0)
            wps = gpool.tile([128, TB * BC], f32, tag="gp", name="warm_ps")
            NWARM = 10
            for i in range(NWARM):
                nc.tensor.matmul(
                    wps[:],
                    warm[:, ds(0, 128)],
                    warm[:],
                    start=(i == 0),
                    stop=(i == NWARM - 1),
                )
            nc.scalar.copy(warm[:], wps[:])  # consume so the tiles are live

            # --- chunk-major pipeline: GEMM (PE) -> copy (ACT) -> scan (DVE) ---
            chunk_order = list(range(NCH - 1, 0, -1)) + [0]
            for g in chunk_order:
                kg = windows[g]
                nbg = kg // TB
                fb = NBLK - nbg
                if g == 0:
                    u_t = ubig.tile([128, BC * kg], f32, tag="u0", name="u_g0")
                    u3 = u_t[:].rearrange("p (b t) -> p b t", b=BC)
                else:
                    # per-batch width kg+2: the last two columns are the BIG
                    # separator pair that resets the fused scan between rows
                    u_t = usmall.tile(
                        [128, BC * (USMALL_W + 2)], f32, tag="us", name=f"u_g{g}"
                    )
                    u3 = u_t[:, ds(0, BC * (kg + 2))].rearrange(
                        "p (b t) -> p b t", b=BC
                    )
                    nc.gpsimd.memset(u3[:, :, ds(kg, 2)], 1.0e30)
                # chunk 0 consumes blocks in ascending (scan) order so each
                # GEMM run's segment scan chains off the previous one; other
                # chunks take newest-first (their X arrives first).
                blocks = (
                    list(range(fb, NBLK))
                    if g == 0
                    else list(range(NBLK - 1, fb - 1, -1))
                )
                def emit_gemm_copy(run):
                    ps = {
                        kb: gpool.tile([128, TB * BC], f32, tag="gp", name=f"gp_{g}_{kb}")
                        for kb in run
                    }
                    for ic in range(NI):
                        for kb in run:
                            nc.tensor.matmul(
                                ps[kb][:],
                                wiht_t[:, ds(g * NI * 128 + ic * 128, 128)],
                                xt[kb][:, ds(ic * TB * BC, TB * BC)],
                                start=(ic == 0),
                                stop=(ic == NI - 1),
                            )
                    for kb in run:
                        j = kb - fb
                        dst = u3[:, :, ds(j * TB, TB)]
                        src = ps[kb][:].rearrange("p (b t) -> p b t", b=BC)
                        nc.scalar.copy(dst, src)

                def emit_seg_scans(first_blk, n_blk):
                    # segment scan seeded by the previous segment's last
                    # element per (lane, b); first segment seeds with zero
                    seg0 = (first_blk - fb) * TB
                    seg = n_blk * TB
                    scn_s = scn_t[:, ds(int(offs[g]) + seg0, seg)]
                    for b in range(BC):
                        ap = u_t[:, ds(b * kg + seg0, seg)]
                        if seg0 == 0:
                            nc.vector._custom_dve(SCAN_OP, out=ap, in0=ap, in1=scn_s)
                        else:
                            seed = u_t[:, ds(b * kg + seg0 - 1, 1)]
                            nc.vector._custom_dve(
                                SCAN_SEED_OP, out=ap, in0=ap, in1=scn_s, s0=seed
                            )

                if g == 0 and len(blocks) >= R + 6 and (len(blocks) - 4 - R) % 2 == 0:
                    # One R-run, then 2-block runs (so each pair's scan rides
                    # right behind its X arrival), with the last 4 blocks
                    # [w, x, y, z] GEMMed as [x, y, z] (X already resident)
                    # then [w] (the last DMA arrival) and scanned as a single
                    # 4-block segment: the post-DMA tail is G+C of w plus one
                    # segment scan.
                    n = len(blocks)
                    runs_scan = [blocks[:R]] + [
                        blocks[i : i + 2] for i in range(R, n - 4, 2)
                    ]
                    for run in runs_scan:
                        emit_gemm_copy(run)
                        emit_seg_scans(run[0], len(run))
                    emit_gemm_copy(blocks[n - 3 :])
                    emit_gemm_copy(blocks[n - 4 : n - 3])
                    emit_seg_scans(blocks[n - 4], 4)
                elif g == 0:
                    full, left = [], list(blocks)
                    while len(left) > 4:
                        full.append(left[:R])
                        left = left[R:]
                    for run in full:
                        emit_gemm_copy(run)
                        emit_seg_scans(run[0], len(run))
                    if len(left) > 1:
                        emit_gemm_copy(left[1:])
                        emit_gemm_copy(left[:1])
                        emit_seg_scans(left[0], 1)
                        emit_seg_scans(left[1], len(left) - 1)
                    else:
                        emit_gemm_copy(left)
                        emit_seg_scans(left[0], 1)
                else:
                    for rs in range(0, len(blocks), R):
                        emit_gemm_copy(blocks[rs : rs + R])
                if g != 0:
                    # one fused scan over all batch rows: the (-1, +1)-scaled
                    # BIG separator pair exactly zeroes the state between rows
                    scn_g = (
                        scn_t[:, ds(int(offs[g]), kg + 2)]
                        .rearrange("p (o t) -> p o t", o=1)
                        .broadcast_to([128, BC, kg + 2])
                    )
                    nc.vector._custom_dve(SCAN_OP, out=u3, in0=u3, in1=scn_g)
                # h_final = last scan element per (lane, b) -> bf16
                hsrc = u3[:, :, kg - 1]
                nc.vector.tensor_copy(h_all[:, ds(g * BC, BC)], hsrc)

            # --- final projection: Y = h^T @ WHOT + bias ---
            # (emitted after all main-GEMM matmuls so no PE-FIFO stall; chunk 0
            # last, so the tail after its scan is a single matmul)
            psy = fpool.tile([BC, O], f32, tag="fy", name="psy")
            for i, g in enumerate(chunk_order):
                nc.tensor.matmul(
                    psy[:],
                    h_all[:, ds(g * BC, BC)],
                    whot_t[:, ds(g * O, O)],
                    start=(i == 0),
                    stop=(i == NCH - 1),
                )
            y_t = ypool.tile([BC, O], f32, tag="y", name="y_t")
            nc.vector.tensor_tensor(y_t[:], psy[:], bias_t[:], mybir.AluOpType.add)
            nc.sync.dma_start(Y, y_t[:])
    nc.compile()
    return nc


def _get_program(windows, offs_total):
    key = (
        windows,
        os.environ.get("DIAG_R"),
        os.environ.get("DIAG_LN"),
    )
    if key not in _CACHE:
        _CACHE[key] = _build(windows, offs_total)
    return _CACHE[key]


def _ensure_ntff_hook():
    """Provide antenv.axon_hooks (absent in this image) so trace=True works."""
    import sys
    import types

    if "antenv.axon_hooks" in sys.modules:
        return True
    try:
        import antenv

        mod = types.ModuleType("antenv.axon_hooks")
        mod._hook = None

        def set_axon_ntff_profile_hook(h):
            mod._hook = h

        def get_axon_ntff_profile_hook():
            return mod._hook

        mod.set_axon_ntff_profile_hook = set_axon_ntff_profile_hook
        mod.get_axon_ntff_profile_hook = get_axon_ntff_profile_hook
        sys.modules["antenv.axon_hooks"] = mod
        antenv.axon_hooks = mod

        from trn_agent_boot.trn_boot import _ntff_profile_via_ctypes

        hook = _ntff_profile_via_ctypes("/opt/axon/libaxon_pjrt.so")
        mod.set_axon_ntff_profile_hook(hook)
        return hook is not None
    except Exception:
        return False


def kernel(X, W_ih, hh, W_ho, b_ho):
    import ml_dtypes

    from concourse import bass_utils

    X = np.asarray(X, dtype=np.float32)
    W_ih = np.asarray(W_ih, dtype=np.float32)
    hh = np.asarray(hh, dtype=np.float32)
    W_ho = np.asarray(W_ho, dtype=np.float32)
    b_ho = np.asarray(b_ho, dtype=np.float32)

    plan = _make_plan(hh)
    perm = plan["perm"]
    nc = _get_program(plan["windows"], int(plan["offs"][-1]))

    bf = ml_dtypes.bfloat16
    # WIHT [128, NCH*NI*128]: line p = [g, ic, hsub], W_ih[h=g*128+hsub, i=ic*128+p]
    wiht = np.ascontiguousarray(
        W_ih[perm].T.reshape(NI, 128, NCH, 128).transpose(1, 2, 0, 3).reshape(128, -1)
    ).astype(bf)
    # WHOT [128, NCH*O]: line p = [g, o] with value W_ho[o, h=g*128+p]
    whot = np.ascontiguousarray(
        W_ho[:, perm].T.reshape(NCH, 128, O).transpose(1, 0, 2).reshape(128, NCH * O)
    ).astype(bf)
    bias = np.tile(b_ho[None, :], (BC, 1)).astype(np.float32)

    common = {
        "WIHT": wiht,
        "WHOT": whot,
        "BIAS": bias,
        "SCN": plan["SCN"].astype(bf),
    }
    in_maps = []
    for m in range(NCORES):
        im = dict(common)
        xm = X[:, m * BC : (m + 1) * BC, :]  # [S, BC, I]
        # device layout [NBLK, 128(i-in-chunk), (ic, b, tau)]
        xt = xm.transpose(2, 1, 0).reshape(NI, 128, BC, NBLK, TB)
        xt = np.ascontiguousarray(xt.transpose(3, 1, 0, 2, 4)).reshape(
            NBLK, 128, NI * BC * TB
        )
        im["X"] = xt.astype(bf)
        in_maps.append(im)

    trace = bool(int(os.environ.get("DIAG_TRACE", "0")))
    if trace:
        trace = _ensure_ntff_hook()
    res = None
    for attempt in range(3):
        try:
            res = bass_utils.run_bass_kernel_spmd(
                nc,
                in_maps,
                core_ids=list(range(NCORES)),
                trace=trace,
                tmpdir=os.environ.get("DIAG_TRACE_DIR") or None,
            )
            break
        except Exception:
            if attempt == 2:
                raise
            trace = False  # retry without profiling
    if res.exec_time_ns is not None:
        kernel.last_exec_time_ns = res.exec_time_ns
        kernel.last_mean_exec_time_ns = res.mean_exec_time_ns
    Yfull = np.concatenate([r["Y"] for r in res.results], axis=0)
    return Yfull


kernel.last_exec_time_ns = None
kernel.last_mean_exec_time_ns = None
